# revision 31
# baseline (speedup 1.0000x reference)
"""GatedPooling Trainium2 kernel (8-core SPMD, data-parallel over batch).

reference math:
    w      = entmax_bisect(attn_scores, alpha=2, dim=T)          # (B, T, 1)
    gate   = sigmoid(x @ gate_w.T + gate_b)                      # (B, T, D)
    pooled = sum_t w * (x * gate)                                # (B, D)

Device layout (per core, NB = B/8 = 4 batches):
  * feature-major: xT[d, t] tiles so the D-contraction matmul needs no
    on-chip transpose (host supplies x transposed + gate_w transposed —
    layout marshaling only; all FLOPs stay on device).
  * fp16 on the matmul + elementwise path: fp32 matmul runs LOW_HIGH
    double-pass on the PE (measured 2x instructions at half rate), and
    fp32 tensor_tensor on DVE is 1 elem/lane/cycle while 16-bit packs
    2x. fp16's 10 mantissa bits keep the absmax-relative error ~4e-4.
    PSUM accumulation and all pooling/entmax accumulators stay fp32.
  * S^T[e, t] = wT[d, e]^T @ xT[d, t] accumulated over 8 d-tiles in a
    two-bank [128, 1024] PSUM tile (two 8-matmul accumulation groups).
  * ACT drains PSUM with fused per-partition bias + sigmoid -> fp16.
  * DVE: gate *= w128, then fused (gate * xT) multiply whose fp32
    accum_out lands directly in the pooled output column.
  * entmax bisection in fp32: the relu+row-sum runs on ACT (fused bias
    AP + accum_out), tiny compare/update ops on DVE; the attn weights
    are partition-broadcast via a DRAM-bounce stride-0 DMA.
"""

import sys

if "/opt/trn_rl_repo" not in sys.path:
    sys.path.insert(0, "/opt/trn_rl_repo")

import numpy as np

import concourse.bacc as bacc
import concourse.tile as tile
from concourse import mybir
from concourse.bass_utils import run_bass_kernel_spmd
from concourse.masks import make_identity

N_CORES = 8
B, T, D = 32, 1024, 1024
NB = B // N_CORES          # batches per core
P = 128                    # partitions
ND = D // P                # d tiles (contraction)
NE = D // P                # e tiles (gate features)
TCH = 512                  # matmul free-dim chunk = one fp32 PSUM bank
NTC = T // TCH
N_ITER = 24                # bisection iters (tau err <= dm0*2^-24 ~ 6e-8)
DM0 = 1.0 - 1.0 / T        # tau_hi - tau_lo, data-independent for alpha=2

F32 = mybir.dt.float32
F16 = mybir.dt.float16
ALU = mybir.AluOpType
AFT = mybir.ActivationFunctionType

_CACHE = {}

# Most recent BassKernelResults (test.py reads exec_time_ns when
# BASS_TRACE is set).
LAST_RESULTS = None


def _build():
    nc = bacc.Bacc("TRN2", target_bir_lowering=False, debug=False,
                   num_devices=N_CORES)
    xt_d = nc.dram_tensor("xt", [NB, D, T], F16, kind="ExternalInput")
    wt_d = nc.dram_tensor("wt", [D, D], F16, kind="ExternalInput")
    bias_d = nc.dram_tensor("bias", [D], F32, kind="ExternalInput")
    sc_d = nc.dram_tensor("scores", [NB, T], F32, kind="ExternalInput")
    out_d = nc.dram_tensor("out", [NB, D], F32, kind="ExternalOutput")

    with tile.TileContext(nc) as tc:
        with (
            tc.tile_pool(name="weights", bufs=1) as wpool,
            tc.tile_pool(name="xtp", bufs=4) as xpool,
            tc.tile_pool(name="gw", bufs=12) as gpool,
            tc.tile_pool(name="small", bufs=1) as spool,
            tc.tile_pool(name="iter", bufs=2) as ipool,
            tc.tile_pool(name="psum", bufs=4, space="PSUM") as ppool,
            tc.tile_pool(name="dram", bufs=1, space="DRAM") as dpool,
        ):
            # ---- entmax bisection, entirely on DVE ---------------------
            # (keeping ACT free to drain PSUM: a serial ACT<->DVE entmax
            # chain was measured starving the sigmoid drains for ~37us)
            X = spool.tile([NB, T], F32)
            nc.sync.dma_start(out=X, in_=sc_d[:, :])
            zeros = spool.tile([NB, T], F32)
            nc.vector.memset(zeros, 0.0)
            mx = spool.tile([NB, 1], F32)
            nc.vector.reduce_max(mx, X, axis=mybir.AxisListType.X)
            # ntau = -(tau_lo) = 1 - max
            ntau = spool.tile([NB, 1], F32)
            nc.vector.tensor_scalar(ntau, mx, -1.0, 1.0, ALU.mult, ALU.add)
            p_scr = spool.tile([NB, T], F32)
            r = spool.tile([NB, 1], F32)
            # p = max(X - tau, 0) with fused row-sum in accum_out
            nc.vector.scalar_tensor_tensor(p_scr, X, ntau, zeros, ALU.add,
                                           ALU.max, accum_out=r)
            flo = spool.tile([NB, 1], F32)
            nc.vector.tensor_scalar_add(flo, r, -1.0)

            dm = DM0
            for _ in range(N_ITER):
                dm *= 0.5
                ntau_m = ipool.tile([NB, 1], F32, tag="ntaum")
                nc.vector.tensor_scalar_add(ntau_m, ntau, -dm)
                nc.vector.scalar_tensor_tensor(p_scr, X, ntau_m, zeros,
                                               ALU.add, ALU.max, accum_out=r)
                # c = (sum - 1) * f_lo ;  tau_lo += dm where c >= 0
                c = ipool.tile([NB, 1], F32, tag="c")
                nc.vector.scalar_tensor_tensor(c, r, -1.0, flo, ALU.add,
                                               ALU.mult)
                step = ipool.tile([NB, 1], F32, tag="step")
                nc.vector.tensor_scalar(step, c, 0.0, -dm, ALU.is_ge,
                                        ALU.mult)
                nc.vector.tensor_add(ntau, ntau, step)

            rec = spool.tile([NB, 1], F32)
            nc.vector.reciprocal(rec, r)
            wn = spool.tile([NB, T], F16)
            nc.vector.tensor_scalar_mul(wn, p_scr, rec)

            # broadcast each batch's weights across all 128 partitions via
            # a DRAM bounce + stride-0 partition-broadcast DMA read
            wdram = dpool.tile([NB, T], F16)
            nc.sync.dma_start(out=wdram, in_=wn)
            w128 = []
            for b in range(NB):
                wb = spool.tile([P, T], F16, tag=f"w128_{b}",
                                name=f"w128_{b}")
                nc.sync.dma_start(out=wb,
                                  in_=wdram[b:b + 1, :].to_broadcast([P, T]))
                w128.append(wb)

            # ---- main gate matmul + pooling ----------------------------
            # few big DMAs: the per-dma_start issue cost (~0.65us on the
            # sync sequencer) was serializing 55 issues and starving the
            # PE for the first ~30us. wt comes in two halves so the first
            # accumulation group can start early; all 4 batches of xT are
            # SBUF-resident (16KB/partition each in fp16).
            wt_sb = wpool.tile([P, ND, D], F16)
            wt_src = wt_d.ap().rearrange("(dt p) e -> p dt e", p=P)
            xt_sb = []
            xt_srcs = []
            for b in range(NB):
                xt_sb.append(xpool.tile([P, ND, T], F16, tag="xt",
                                        name=f"xt{b}"))
                xt_srcs.append(xt_d[b].rearrange("(dt p) t -> p dt t", p=P))
            # wt and batch-0 xT arrive as interleaved chunks (fine-grained
            # at the head) so the first accumulation groups start early
            q = 0
            for step in (1, 1, 2, 2, 2):
                sl = slice(q, q + step)
                nc.sync.dma_start(out=wt_sb[:, sl, :], in_=wt_src[:, sl, :])
                nc.sync.dma_start(out=xt_sb[0][:, sl, :],
                                  in_=xt_srcs[0][:, sl, :])
                q += step
            bias_sb = spool.tile([P, NE], F32)
            nc.sync.dma_start(
                out=bias_sb, in_=bias_d.ap().rearrange("(e p) -> p e", p=P))
            for b in range(1, NB):
                nc.sync.dma_start(out=xt_sb[b][:, 0:ND // 2, :],
                                  in_=xt_srcs[b][:, 0:ND // 2, :])
                nc.sync.dma_start(out=xt_sb[b][:, ND // 2:, :],
                                  in_=xt_srcs[b][:, ND // 2:, :])
            # pooled columns land in one [128, NE*NB] tile; a single PE
            # transpose at the end turns them into 512B-contiguous DRAM
            # rows (the naive per-column DMA was 16us of 4B-scatter)
            pooled = spool.tile([P, NE * NB], F32)
            identity = spool.tile([P, P], F32)
            make_identity(nc, identity)
            for b in range(NB):
                xt_b = xt_sb[b]
                for et in range(NE):
                    ps = ppool.tile([P, T], F32, tag="ps", bufs=3)
                    for tci in range(NTC):
                        tsl = slice(tci * TCH, (tci + 1) * TCH)
                        for dt in range(ND):
                            nc.tensor.matmul(
                                ps[:, tsl],
                                lhsT=wt_sb[:, dt, et * P:(et + 1) * P],
                                rhs=xt_b[:, dt, tsl],
                                start=(dt == 0),
                                stop=(dt == ND - 1),
                            )
                    g = gpool.tile([P, T], F16, tag="g")
                    nc.scalar.activation(g, ps, AFT.Sigmoid,
                                         bias=bias_sb[:, et:et + 1],
                                         scale=1.0)
                    nc.vector.tensor_mul(g, g, w128[b])
                    # (g * 1.0) * xT with fp32 accum -> pooled column
                    # (tensor_tensor_reduce would fuse this but dies with a
                    # runtime INTERNAL error on this stack)
                    col = b * NE + et
                    nc.vector.scalar_tensor_tensor(
                        g, g, 1.0, xt_b[:, et, :], ALU.mult, ALU.mult,
                        accum_out=pooled[:, col:col + 1])
            psum_t = ppool.tile([NE * NB, P], F32, tag="pst", bufs=1)
            nc.tensor.transpose(psum_t, pooled, identity)
            out_t = spool.tile([NE * NB, P], F32)
            nc.vector.tensor_copy(out_t, psum_t)
            nc.sync.dma_start(
                out=out_d.ap().rearrange("b (et p) -> (b et) p", p=P),
                in_=out_t)

    nc.compile()
    return nc


def _get_nc():
    if "nc" not in _CACHE:
        _CACHE["nc"] = _build()
    return _CACHE["nc"]


def kernel(x, attn_scores, gate_w, gate_b):
    global LAST_RESULTS
    nc = _get_nc()
    xt = np.ascontiguousarray(
        np.transpose(np.asarray(x), (0, 2, 1))).astype(np.float16)
    wt = np.ascontiguousarray(np.asarray(gate_w).T).astype(np.float16)
    bias = np.ascontiguousarray(np.asarray(gate_b, dtype=np.float32))
    scores = np.ascontiguousarray(
        np.asarray(attn_scores, dtype=np.float32)[:, :, 0])

    in_maps = []
    for cid in range(N_CORES):
        sl = slice(cid * NB, (cid + 1) * NB)
        in_maps.append({
            "xt": xt[sl],
            "wt": wt,
            "bias": bias,
            "scores": scores[sl],
        })
    res = run_bass_kernel_spmd(nc, in_maps, list(range(N_CORES)))
    LAST_RESULTS = res
    return np.concatenate([res.results[c]["out"] for c in range(N_CORES)],
                          axis=0)


# revision 32
# speedup vs baseline: 1.0120x; 1.0120x over previous
"""GatedPooling Trainium2 kernel (8-core SPMD, data-parallel over batch).

reference math:
    w      = entmax_bisect(attn_scores, alpha=2, dim=T)          # (B, T, 1)
    gate   = sigmoid(x @ gate_w.T + gate_b)                      # (B, T, D)
    pooled = sum_t w * (x * gate)                                # (B, D)

Device layout (per core, NB = B/8 = 4 batches):
  * feature-major: xT[d, t] tiles so the D-contraction matmul needs no
    on-chip transpose (host supplies x transposed + gate_w transposed —
    layout marshaling only; all FLOPs stay on device).
  * fp16 on the matmul + elementwise path: fp32 matmul runs LOW_HIGH
    double-pass on the PE (measured 2x instructions at half rate), and
    fp32 tensor_tensor on DVE is 1 elem/lane/cycle while 16-bit packs
    2x. fp16's 10 mantissa bits keep the absmax-relative error ~4e-4.
    PSUM accumulation and all pooling/entmax accumulators stay fp32.
  * S^T[e, t] = wT[d, e]^T @ xT[d, t] accumulated over 8 d-tiles in a
    two-bank [128, 1024] PSUM tile (two 8-matmul accumulation groups).
  * ACT drains PSUM with fused per-partition bias + sigmoid -> fp16.
  * DVE: gate *= w128, then fused (gate * xT) multiply whose fp32
    accum_out lands directly in the pooled output column.
  * entmax bisection in fp32, entirely on DVE (fused relu+row-sum via
    scalar_tensor_tensor accum_out) so the serial chain never blocks
    ACT's PSUM drains; the attn weights are partition-broadcast via a
    DRAM-bounce stride-0 DMA.
"""

import sys

if "/opt/trn_rl_repo" not in sys.path:
    sys.path.insert(0, "/opt/trn_rl_repo")

import numpy as np

import concourse.bacc as bacc
import concourse.tile as tile
from concourse import mybir
from concourse.bass_utils import run_bass_kernel_spmd
from concourse.masks import make_identity

N_CORES = 8
B, T, D = 32, 1024, 1024
NB = B // N_CORES          # batches per core
P = 128                    # partitions
ND = D // P                # d tiles (contraction)
NE = D // P                # e tiles (gate features)
TCH = 512                  # matmul free-dim chunk = one fp32 PSUM bank
NTC = T // TCH
N_ITER = 24                # bisection iters (tau err <= dm0*2^-24 ~ 6e-8)
DM0 = 1.0 - 1.0 / T        # tau_hi - tau_lo, data-independent for alpha=2

F32 = mybir.dt.float32
F16 = mybir.dt.float16
ALU = mybir.AluOpType
AFT = mybir.ActivationFunctionType

_CACHE = {}

# Most recent BassKernelResults (test.py reads exec_time_ns when
# BASS_TRACE is set).
LAST_RESULTS = None


def _build():
    nc = bacc.Bacc("TRN2", target_bir_lowering=False, debug=False,
                   num_devices=N_CORES)
    xt_d = nc.dram_tensor("xt", [NB, D, T], F16, kind="ExternalInput")
    wt_d = nc.dram_tensor("wt", [D, D], F16, kind="ExternalInput")
    bias_d = nc.dram_tensor("bias", [D], F32, kind="ExternalInput")
    sc_d = nc.dram_tensor("scores", [NB, T], F32, kind="ExternalInput")
    out_d = nc.dram_tensor("out", [NB, D], F32, kind="ExternalOutput")

    with tile.TileContext(nc) as tc:
        with (
            tc.tile_pool(name="weights", bufs=1) as wpool,
            tc.tile_pool(name="xtp", bufs=4) as xpool,
            tc.tile_pool(name="gw", bufs=12) as gpool,
            tc.tile_pool(name="small", bufs=1) as spool,
            tc.tile_pool(name="iter", bufs=2) as ipool,
            tc.tile_pool(name="psum", bufs=4, space="PSUM") as ppool,
            tc.tile_pool(name="dram", bufs=1, space="DRAM") as dpool,
        ):
            # ---- entmax bisection, entirely on DVE ---------------------
            # (keeping ACT free to drain PSUM: a serial ACT<->DVE entmax
            # chain was measured starving the sigmoid drains for ~37us)
            X = spool.tile([NB, T], F32)
            nc.sync.dma_start(out=X, in_=sc_d[:, :])
            zeros = spool.tile([NB, T], F32)
            nc.vector.memset(zeros, 0.0)
            mx = spool.tile([NB, 1], F32)
            nc.vector.reduce_max(mx, X, axis=mybir.AxisListType.X)
            # ntau = -(tau_lo) = 1 - max
            ntau = spool.tile([NB, 1], F32)
            nc.vector.tensor_scalar(ntau, mx, -1.0, 1.0, ALU.mult, ALU.add)
            p_scr = spool.tile([NB, T], F32)
            r = spool.tile([NB, 1], F32)
            # p = max(X - tau, 0) with fused row-sum in accum_out
            nc.vector.scalar_tensor_tensor(p_scr, X, ntau, zeros, ALU.add,
                                           ALU.max, accum_out=r)
            flo = spool.tile([NB, 1], F32)
            nc.vector.tensor_scalar_add(flo, r, -1.0)

            dm = DM0
            for _ in range(N_ITER):
                dm *= 0.5
                ntau_m = ipool.tile([NB, 1], F32, tag="ntaum")
                nc.vector.tensor_scalar_add(ntau_m, ntau, -dm)
                nc.vector.scalar_tensor_tensor(p_scr, X, ntau_m, zeros,
                                               ALU.add, ALU.max, accum_out=r)
                # c = (sum - 1) * f_lo ;  tau_lo += dm where c >= 0
                c = ipool.tile([NB, 1], F32, tag="c")
                nc.vector.scalar_tensor_tensor(c, r, -1.0, flo, ALU.add,
                                               ALU.mult)
                step = ipool.tile([NB, 1], F32, tag="step")
                nc.vector.tensor_scalar(step, c, 0.0, -dm, ALU.is_ge,
                                        ALU.mult)
                nc.vector.tensor_add(ntau, ntau, step)

            rec = spool.tile([NB, 1], F32)
            nc.vector.reciprocal(rec, r)
            wn = spool.tile([NB, T], F16)
            nc.vector.tensor_scalar_mul(wn, p_scr, rec)

            # broadcast each batch's weights across all 128 partitions via
            # a DRAM bounce + stride-0 partition-broadcast DMA read
            wdram = dpool.tile([NB, T], F16)
            nc.sync.dma_start(out=wdram, in_=wn)
            w128 = []
            for b in range(NB):
                wb = spool.tile([P, T], F16, tag=f"w128_{b}",
                                name=f"w128_{b}")
                nc.sync.dma_start(out=wb,
                                  in_=wdram[b:b + 1, :].to_broadcast([P, T]))
                w128.append(wb)

            # ---- main gate matmul + pooling ----------------------------
            # few big DMAs: the per-dma_start issue cost (~0.65us on the
            # sync sequencer) was serializing 55 issues and starving the
            # PE for the first ~30us. wt comes in two halves so the first
            # accumulation group can start early; all 4 batches of xT are
            # SBUF-resident (16KB/partition each in fp16).
            wt_sb = wpool.tile([P, ND, D], F16)
            wt_src = wt_d.ap().rearrange("(dt p) e -> p dt e", p=P)
            xt_sb = []
            xt_srcs = []
            for b in range(NB):
                xt_sb.append(xpool.tile([P, ND, T], F16, tag="xt",
                                        name=f"xt{b}"))
                xt_srcs.append(xt_d[b].rearrange("(dt p) t -> p dt t", p=P))
            # wt and batch-0 xT arrive as interleaved chunks (fine-grained
            # at the head) so the first accumulation groups start early
            q = 0
            for step in (1, 1, 2, 2, 2):
                sl = slice(q, q + step)
                nc.sync.dma_start(out=wt_sb[:, sl, :], in_=wt_src[:, sl, :])
                nc.sync.dma_start(out=xt_sb[0][:, sl, :],
                                  in_=xt_srcs[0][:, sl, :])
                q += step
            bias_sb = spool.tile([P, NE], F32)
            nc.sync.dma_start(
                out=bias_sb, in_=bias_d.ap().rearrange("(e p) -> p e", p=P))
            for b in range(1, NB):
                nc.sync.dma_start(out=xt_sb[b][:, 0:ND // 2, :],
                                  in_=xt_srcs[b][:, 0:ND // 2, :])
                nc.sync.dma_start(out=xt_sb[b][:, ND // 2:, :],
                                  in_=xt_srcs[b][:, ND // 2:, :])
            # pooled columns land in one [128, NE*NB] tile; a single PE
            # transpose at the end turns them into 512B-contiguous DRAM
            # rows (the naive per-column DMA was 16us of 4B-scatter)
            pooled = spool.tile([P, NE * NB], F32)
            identity = spool.tile([P, P], F32)
            make_identity(nc, identity)
            for b in range(NB):
                xt_b = xt_sb[b]
                for et in range(NE):
                    ps = ppool.tile([P, T], F32, tag="ps", bufs=3)
                    for tci in range(NTC):
                        tsl = slice(tci * TCH, (tci + 1) * TCH)
                        for dt in range(ND):
                            nc.tensor.matmul(
                                ps[:, tsl],
                                lhsT=wt_sb[:, dt, et * P:(et + 1) * P],
                                rhs=xt_b[:, dt, tsl],
                                start=(dt == 0),
                                stop=(dt == ND - 1),
                            )
                    g = gpool.tile([P, T], F16, tag="g")
                    nc.scalar.activation(g, ps, AFT.Sigmoid,
                                         bias=bias_sb[:, et:et + 1],
                                         scale=1.0)
                    nc.vector.tensor_mul(g, g, w128[b])
                    # (g * 1.0) * xT with fp32 accum -> pooled column
                    # (tensor_tensor_reduce would fuse this but dies with a
                    # runtime INTERNAL error on this stack)
                    col = b * NE + et
                    nc.vector.scalar_tensor_tensor(
                        g, g, 1.0, xt_b[:, et, :], ALU.mult, ALU.mult,
                        accum_out=pooled[:, col:col + 1])
            psum_t = ppool.tile([NE * NB, P], F32, tag="pst", bufs=1)
            nc.tensor.transpose(psum_t, pooled, identity)
            out_t = spool.tile([NE * NB, P], F32)
            nc.vector.tensor_copy(out_t, psum_t)
            nc.sync.dma_start(
                out=out_d.ap().rearrange("b (et p) -> (b et) p", p=P),
                in_=out_t)

    nc.compile()
    return nc


def _get_nc():
    if "nc" not in _CACHE:
        _CACHE["nc"] = _build()
    return _CACHE["nc"]


def kernel(x, attn_scores, gate_w, gate_b):
    global LAST_RESULTS
    nc = _get_nc()
    xt = np.ascontiguousarray(
        np.transpose(np.asarray(x), (0, 2, 1))).astype(np.float16)
    wt = np.ascontiguousarray(np.asarray(gate_w).T).astype(np.float16)
    bias = np.ascontiguousarray(np.asarray(gate_b, dtype=np.float32))
    scores = np.ascontiguousarray(
        np.asarray(attn_scores, dtype=np.float32)[:, :, 0])

    in_maps = []
    for cid in range(N_CORES):
        sl = slice(cid * NB, (cid + 1) * NB)
        in_maps.append({
            "xt": xt[sl],
            "wt": wt,
            "bias": bias,
            "scores": scores[sl],
        })
    res = run_bass_kernel_spmd(nc, in_maps, list(range(N_CORES)))
    LAST_RESULTS = res
    return np.concatenate([res.results[c]["out"] for c in range(N_CORES)],
                          axis=0)
